# revision 13
# baseline (speedup 1.0000x reference)
"""Trainium2 kernel for CustomContextEncoderForQG.

Full on-device pipeline:
- LSTM layer NEFF (runs SPMD on 2 cores, one direction per core, the
  backward direction is fed time-reversed inputs so the program is uniform):
  input projection (xp = Wih @ x + b, masked) into a DRAM scratch, then the
  512-step recurrence with gates in [2560(part), 16(batch)] layout.
- Attention NEFF (8 cores, 2 sequences/core): QKV projections with q,k in
  transposed [d, s] layout and v in natural [s, d] layout, per-head
  max-free softmax with the additive mask as a per-partition ACT bias,
  normalization via a K=1 broadcast matmul, residual add fused.
Host glue handles the between-layer reversal/masking and final assembly.
Falls back to a pure numpy implementation on any device failure.
"""

import sys
import numpy as np

sys.path.insert(0, "/opt/trn_rl_repo")

import ml_dtypes

BF16 = ml_dtypes.bfloat16

B, S, D_MODEL, H, NHEADS = 16, 512, 768, 640, 10
D_ATT = 2 * H  # 1280
HEAD_DIM = D_ATT // NHEADS  # 128
N_CORES = 8
BPC = B // N_CORES  # 2 sequences per core
H4 = 4 * H  # 2560
NMC = H4 // 128  # 20 gate tiles
NKH = H // 128  # 5 h k-tiles
SCALE = float(1.0 / np.sqrt(HEAD_DIM))

_NC_CACHE = {}
TRACE_LOG = []


# ---------------------------------------------------------------- numpy ref
def _sigmoid(x):
    return 1.0 / (1.0 + np.exp(-x))


def _lstm_dir_np(xp, Whh, lengths, reverse):
    Bs, Ss, H4_ = xp.shape
    Hh = H4_ // 4
    WhhT = np.ascontiguousarray(Whh.T)
    h = np.zeros((Bs, Hh), np.float32)
    c = np.zeros((Bs, Hh), np.float32)
    out = np.zeros((Bs, Ss, Hh), np.float32)
    ts_ = range(Ss - 1, -1, -1) if reverse else range(Ss)
    for t in ts_:
        g = xp[:, t] + h @ WhhT
        i = _sigmoid(g[:, :Hh])
        f = _sigmoid(g[:, Hh : 2 * Hh])
        gg = np.tanh(g[:, 2 * Hh : 3 * Hh])
        o = _sigmoid(g[:, 3 * Hh :])
        c2 = f * c + i * gg
        h2 = o * np.tanh(c2)
        valid = (t < lengths)[:, None]
        h = np.where(valid, h2, h)
        c = np.where(valid, c2, c)
        out[:, t] = np.where(valid, h, 0.0)
    return out


def _bilstm_layer_np(x, Wih, Whh, b, lengths):
    outs = []
    for d, rev in ((0, False), (1, True)):
        xp = x @ Wih[d].T + b[d]
        outs.append(_lstm_dir_np(xp, Whh[d], lengths, rev))
    return np.concatenate(outs, axis=-1)


def _attention_np(h, mask, Wq, bq, Wk, bk, Wv, bv):
    q = (h @ Wq.T + bq).reshape(B, S, NHEADS, HEAD_DIM)
    k = (h @ Wk.T + bk).reshape(B, S, NHEADS, HEAD_DIM)
    v = (h @ Wv.T + bv).reshape(B, S, NHEADS, HEAD_DIM)
    scores = np.einsum("bqhd,bkhd->bhqk", q, k) / np.float32(np.sqrt(HEAD_DIM))
    scores = scores + mask
    scores = scores - scores.max(-1, keepdims=True)
    e = np.exp(scores)
    probs = e / e.sum(-1, keepdims=True)
    ctx = np.einsum("bhqk,bkhd->bqhd", probs, v).reshape(B, S, D_ATT)
    return h + ctx


def _numpy_forward(c_a_embeds, c_mask, c_lengths, Wih0, Whh0, b0, Wih1, Whh1,
                   b1, Wq, bq, Wk, bk, Wv, bv):
    lengths = np.asarray(c_lengths)
    h = _bilstm_layer_np(np.asarray(c_a_embeds, np.float32), np.asarray(Wih0),
                         np.asarray(Whh0), np.asarray(b0), lengths)
    h = _bilstm_layer_np(h, np.asarray(Wih1), np.asarray(Whh1),
                         np.asarray(b1), lengths)
    return _attention_np(h, np.asarray(c_mask, np.float32), np.asarray(Wq),
                         np.asarray(bq), np.asarray(Wk), np.asarray(bk),
                         np.asarray(Wv), np.asarray(bv))


# ------------------------------------------------------------- LSTM builder
def _build_lstm_nc(kc_in):
    """One BiLSTM layer, one direction per core (uniform program).

    Inputs (per core):
      xt    [kc_in, 128, S*16]  bf16  input transposed, (t,b) cols, b fastest
      wiht  [128, kc_in*2560]   bf16  lhsT tiles of input projection
      bias  [128, 20]           f32   combined bias per gate-dim
      vmask [128, S*16]         bf16  1.0 where t valid for that seq else 0.0
      whht  [128, 5*2560]       bf16  lhsT tiles of recurrent weights
    Output:
      y     [128, S*80]         bf16  y[p, tau*80 + hc*16+b] = h_t[hc*128+p, b]
    """
    import concourse.bass as bass
    import concourse.bacc as bacc
    import concourse.mybir as mybir
    from concourse import tile

    fp32 = mybir.dt.float32
    bf16 = mybir.dt.bfloat16
    NB = S * B  # 8192 columns
    NCH = NB // 512  # 16 proj chunks == 32-step recurrence blocks
    TBLK = 32  # recurrence steps per block
    BLKC = NMC * 512  # 10240 xp cols per block, layout [mc, tau, b]

    nc = bacc.Bacc("TRN2", target_bir_lowering=False)
    xt_ext = nc.declare_dram_parameter("xt", [kc_in, 128, NB], bf16, isOutput=False)
    wiht_ext = nc.declare_dram_parameter("wiht", [128, kc_in * H4], bf16, isOutput=False)
    bias_ext = nc.declare_dram_parameter("bias", [128, NMC], fp32, isOutput=False)
    vmask_ext = nc.declare_dram_parameter("vmask", [128, NB], bf16, isOutput=False)
    whht_ext = nc.declare_dram_parameter("whht", [128, NKH * H4], bf16, isOutput=False)
    y_ext = nc.declare_dram_parameter("y", [128, S * 80], bf16, isOutput=True)

    with tile.TileContext(nc) as tc:
        with (
            tc.tile_pool(name="persist", bufs=1) as persist,
            tc.tile_pool(name="xtp", bufs=2) as xtp,
            tc.tile_pool(name="xpsp", bufs=3) as xpsp,
            tc.tile_pool(name="projps", bufs=3, space="PSUM") as projps,
            tc.tile_pool(name="recps", bufs=1, space="PSUM") as recps,
            tc.tile_pool(name="xpbuf", bufs=1) as xpbuf,
            tc.tile_pool(name="ybuf", bufs=1) as ybuf,
            tc.tile_pool(name="work", bufs=2) as work,
            tc.tile_pool(name="dram", bufs=1, space="DRAM") as drampool,
        ):
            # xp scratch: block layout, col = blk*BLKC + mc*512 + tau*16 + b
            # (+2 blocks of padding for the tail prefetch)
            xp_dram = drampool.tile([128, (NCH + 2) * BLKC], fp32, tag="xpd")
            xpr = xp_dram

            wiht = persist.tile([128, kc_in * H4], bf16, tag="wiht")
            nc.sync.dma_start(out=wiht[:], in_=wiht_ext[:, :])
            bias = persist.tile([128, NMC], fp32, tag="bias")
            nc.sync.dma_start(out=bias[:], in_=bias_ext[:, :])
            vmask = persist.tile([128, NB], bf16, tag="vmask")
            nc.sync.dma_start(out=vmask[:], in_=vmask_ext[:, :])
            whht = persist.tile([128, NKH * H4], bf16, tag="whht")
            nc.sync.dma_start(out=whht[:], in_=whht_ext[:, :])
            warm = persist.tile([128, 1], fp32, tag="warm")
            nc.vector.tensor_copy(out=warm[:], in_=bias[:, 0:1])
            warm2 = persist.tile([128, 1], bf16, tag="warm2")
            nc.vector.tensor_copy(out=warm2[:], in_=vmask[:, 0:1])

            # ---------------- projection phase: xp = mask * (Wih @ x + b)
            # Every DMA here is fully contiguous on both sides.
            for nch in range(NCH):
                xts = []
                for kc in range(kc_in):
                    xtt = xtp.tile([128, 512], bf16, tag=f"xt{kc}")
                    nc.sync.dma_start(
                        out=xtt[:], in_=xt_ext[kc, :, nch * 512 : (nch + 1) * 512]
                    )
                    xts.append(xtt)
                for mc in range(NMC):
                    ps = projps.tile([128, 512], fp32, tag="pps")
                    for kc in range(kc_in):
                        nc.tensor.matmul(
                            ps[:],
                            wiht[:, kc * H4 + mc * 128 : kc * H4 + (mc + 1) * 128],
                            xts[kc][:],
                            start=(kc == 0),
                            stop=(kc == kc_in - 1),
                        )
                    xps = xpsp.tile([128, 512], fp32, tag="xps")
                    nc.vector.scalar_tensor_tensor(
                        out=xps[:], in0=ps[:], scalar=bias[:, mc : mc + 1],
                        in1=vmask[:, nch * 512 : (nch + 1) * 512],
                        op0=mybir.AluOpType.add, op1=mybir.AluOpType.mult,
                    )
                    nc.sync.dma_start(
                        out=xp_dram[
                            :, nch * BLKC + mc * 512 : nch * BLKC + (mc + 1) * 512
                        ],
                        in_=xps[:],
                    )

            # ---------------- recurrence phase
            # xp stays f32 for precision; half-block (16-step) double
            # buffering keeps the SBUF footprint at 2 x 20 KiB/partition.
            HBLK = TBLK // 2  # 16 steps per half-block
            HC = NMC * HBLK * 16  # 5120 cols per half-block tile
            X0 = xpbuf.tile([128, HC], fp32, tag="X0")
            X1 = xpbuf.tile([128, HC], fp32, tag="X1")
            yA = ybuf.tile([128, TBLK * 80], bf16, tag="yA")
            cA = ybuf.tile([128, 80], fp32, tag="cA")
            cB = ybuf.tile([128, 80], fp32, tag="cB")
            psA = recps.tile([128, 320], fp32, tag="psA")
            psB = recps.tile([128, 320], fp32, tag="psB")

            def xp_half(bi, j):
                """DRAM view of half-block j of block bi (symbolic bi)."""
                return xpr[:, bass.ts(bi, BLKC)].rearrange(
                    "p (m t) -> p m t", m=NMC
                )[:, :, j * HBLK * 16 : (j + 1) * HBLK * 16]

            nc.vector.memset(yA[:, (TBLK - 1) * 80 : TBLK * 80], 0.0)
            nc.vector.memset(cB[:], 0.0)
            nc.sync.dma_start(out=X0[:], in_=xp_half(0, 0))
            nc.sync.dma_start(out=X1[:], in_=xp_half(0, 1))

            def rec_step(l):
                xsrc = X0 if l < HBLK else X1
                ll = l % HBLK
                # half-block view [p, mc, tau, b] -> step tau=ll: [p, mc, b]
                xp_t = xsrc[:].rearrange(
                    "p (m t b) -> p t m b", m=NMC, t=HBLK
                )[:, ll, :, :]
                if l == 0:
                    hprev = yA[:, (TBLK - 1) * 80 : TBLK * 80]
                else:
                    hprev = yA[:, (l - 1) * 80 : l * 80]
                c_r = cB if l % 2 == 0 else cA
                c_w = cA if l % 2 == 0 else cB
                ps = psA if l % 2 == 0 else psB

                for mc in range(NMC):
                    for kc in range(NKH):
                        nc.tensor.matmul(
                            ps[:, mc * 16 : (mc + 1) * 16],
                            whht[:, kc * H4 + mc * 128 : kc * H4 + (mc + 1) * 128],
                            hprev[:, kc * 16 : (kc + 1) * 16],
                            start=(kc == 0),
                            stop=(kc == NKH - 1),
                        )
                g = work.tile([128, 320], fp32, tag="g")
                nc.vector.tensor_tensor(
                    out=g[:].rearrange("p (m b) -> p m b", m=NMC),
                    in0=ps[:].rearrange("p (m b) -> p m b", m=NMC),
                    in1=xp_t,
                    op=mybir.AluOpType.add,
                )
                a_if = work.tile([128, 160], fp32, tag="aif")
                nc.scalar.activation(
                    out=a_if[:], in_=g[:, 0:160],
                    func=mybir.ActivationFunctionType.Sigmoid,
                )
                a_g = work.tile([128, 80], fp32, tag="ag")
                nc.scalar.activation(
                    out=a_g[:], in_=g[:, 160:240],
                    func=mybir.ActivationFunctionType.Tanh,
                )
                a_o = work.tile([128, 80], fp32, tag="ao")
                nc.scalar.activation(
                    out=a_o[:], in_=g[:, 240:320],
                    func=mybir.ActivationFunctionType.Sigmoid,
                )
                ig = work.tile([128, 80], fp32, tag="ig")
                nc.vector.tensor_tensor(
                    out=ig[:], in0=a_if[:, 0:80], in1=a_g[:],
                    op=mybir.AluOpType.mult,
                )
                fc = work.tile([128, 80], fp32, tag="fc")
                nc.vector.tensor_tensor(
                    out=fc[:], in0=a_if[:, 80:160], in1=c_r[:],
                    op=mybir.AluOpType.mult,
                )
                nc.vector.tensor_tensor(
                    out=c_w[:], in0=ig[:], in1=fc[:], op=mybir.AluOpType.add
                )
                tc2 = work.tile([128, 80], fp32, tag="tc2")
                nc.scalar.activation(
                    out=tc2[:], in_=c_w[:],
                    func=mybir.ActivationFunctionType.Tanh,
                )
                nc.vector.tensor_tensor(
                    out=yA[:, l * 80 : (l + 1) * 80], in0=a_o[:], in1=tc2[:],
                    op=mybir.AluOpType.mult,
                )

            with tc.For_i(0, NCH, 1) as i:
                for l in range(HBLK):
                    rec_step(l)
                nc.sync.dma_start(out=X0[:], in_=xp_half(i + 1, 0))
                nc.sync.dma_start(
                    out=y_ext[:, bass.ts(2 * i, HBLK * 80)], in_=yA[:, 0 : HBLK * 80]
                )
                for l in range(HBLK, TBLK):
                    rec_step(l)
                nc.sync.dma_start(out=X1[:], in_=xp_half(i + 1, 1))
                nc.sync.dma_start(
                    out=y_ext[:, bass.ts(2 * i + 1, HBLK * 80)],
                    in_=yA[:, HBLK * 80 : TBLK * 80],
                )
    nc.finalize()
    return nc


# -------------------------------------------------------- attention builder
def _build_attn_nc():
    """Attention for 2 sequences per core.

    Inputs:
      ht    [2, 10, 128, 512] bf16   h transposed per seq: ht[b,kc,p,s]
      wqt   [128, 10*1280]    bf16   lhsT tiles: col kc*1280+do = Wq.T[kc*128+p, do]
      wkt   [128, 10*1280]    bf16
      wvt   [128, 10*1280]    bf16   rhs tiles for v: col kc*1280+d = Wv.T[kc*128+p, d]
      bqk   [128, 20]         f32    cols 0..9 bq tiles, 10..19 bk tiles
      maskb [128, 8]          f32    col b*4+kt = additive mask for k=kt*128+p
    Output:
      out   [2, 10, 128, 512] f32    out[b,dc,p,q] = result[b, q, dc*128+p]
    """
    import concourse.bass as bass
    import concourse.bacc as bacc
    import concourse.mybir as mybir
    from concourse import tile

    fp32 = mybir.dt.float32
    bf16 = mybir.dt.bfloat16
    NDC = 10
    NST = 4  # 512/128 seq tiles

    from concourse.masks import make_identity

    nc = bacc.Bacc("TRN2", target_bir_lowering=False)
    ht_ext = nc.declare_dram_parameter("ht", [BPC, NDC, 128, S], bf16, isOutput=False)
    wqt_ext = nc.declare_dram_parameter("wqt", [128, NDC * D_ATT], bf16, isOutput=False)
    wkt_ext = nc.declare_dram_parameter("wkt", [128, NDC * D_ATT], bf16, isOutput=False)
    wvt_ext = nc.declare_dram_parameter("wvt", [128, NDC * D_ATT], bf16, isOutput=False)
    bqk_ext = nc.declare_dram_parameter("bqk", [128, 3 * NDC], fp32, isOutput=False)
    maskb_ext = nc.declare_dram_parameter("maskb", [128, BPC * NST], fp32, isOutput=False)
    out_ext = nc.declare_dram_parameter("out", [BPC, S, D_ATT], fp32, isOutput=True)

    with tile.TileContext(nc) as tc:
        with (
            tc.tile_pool(name="persist", bufs=1) as persist,
            tc.tile_pool(name="seqt", bufs=1) as seqt,
            tc.tile_pool(name="pwork", bufs=2) as pwork,
            tc.tile_pool(name="projps", bufs=2, space="PSUM") as projps,
            tc.tile_pool(name="scps", bufs=1, space="PSUM") as scps,
            tc.tile_pool(name="ctxps", bufs=2, space="PSUM") as ctxps,
            tc.tile_pool(name="smps", bufs=1, space="PSUM") as smps,
            tc.tile_pool(name="rbps", bufs=1, space="PSUM") as rbps,
        ):
            ident = persist.tile([128, 128], fp32, tag="ident")
            make_identity(nc, ident[:])
            wqt = persist.tile([128, NDC * D_ATT], bf16, tag="wqt")
            nc.sync.dma_start(out=wqt[:], in_=wqt_ext[:, :])
            wkt = persist.tile([128, NDC * D_ATT], bf16, tag="wkt")
            nc.sync.dma_start(out=wkt[:], in_=wkt_ext[:, :])
            wvt = persist.tile([128, NDC * D_ATT], bf16, tag="wvt")
            nc.sync.dma_start(out=wvt[:], in_=wvt_ext[:, :])
            bqk = persist.tile([128, 3 * NDC], fp32, tag="bqk")
            nc.sync.dma_start(out=bqk[:], in_=bqk_ext[:, :])
            maskb = persist.tile([128, BPC * NST], fp32, tag="maskb")
            nc.sync.dma_start(out=maskb[:], in_=maskb_ext[:, :])
            warm = persist.tile([128, 1], fp32, tag="warm")
            nc.vector.tensor_copy(out=warm[:], in_=bqk[:, 0:1])
            warm2 = persist.tile([128, 1], fp32, tag="warm2")
            nc.scalar.copy(out=warm2[:], in_=maskb[:, 0:1])
            ones1 = persist.tile([128, 1], bf16, tag="ones1")
            nc.vector.memset(ones1[:], 1.0)
            onesr = persist.tile([1, 128], bf16, tag="onesr")
            nc.vector.memset(onesr[:], 1.0)

            for b in range(BPC):
                hts = []
                for kc in range(NDC):
                    htt = seqt.tile([128, S], bf16, tag=f"ht{kc}")
                    nc.sync.dma_start(out=htt[:], in_=ht_ext[b, kc, :, :])
                    hts.append(htt)

                # q,k transposed [do, s]
                qkt = {}
                for name, wt, boff in (("q", wqt, 0), ("k", wkt, NDC)):
                    tiles = []
                    for dc in range(NDC):
                        ps = projps.tile([128, S], fp32, tag="pps")
                        for kc in range(NDC):
                            nc.tensor.matmul(
                                ps[:],
                                wt[:, kc * D_ATT + dc * 128 : kc * D_ATT + (dc + 1) * 128],
                                hts[kc][:],
                                start=(kc == 0),
                                stop=(kc == NDC - 1),
                            )
                        ob = seqt.tile([128, S], bf16, tag=f"{name}T{dc}")
                        nc.vector.tensor_scalar_add(
                            out=ob[:], in0=ps[:],
                            scalar1=bqk[:, boff + dc : boff + dc + 1],
                        )
                        tiles.append(ob)
                    qkt[name] = tiles

                # v natural [s, d]: 4 seq tiles x 1280
                vts = []
                for st in range(NST):
                    vt = seqt.tile([128, D_ATT], bf16, tag=f"v{st}")
                    for d0 in (0, 512, 1024):
                        dn = min(512, D_ATT - d0)
                        ps = projps.tile([128, 512], fp32, tag="pps")
                        for kc in range(NDC):
                            nc.tensor.matmul(
                                ps[:, 0:dn],
                                hts[kc][:, st * 128 : (st + 1) * 128],
                                wvt[:, kc * D_ATT + d0 : kc * D_ATT + d0 + dn],
                                start=(kc == 0),
                                stop=(kc == NDC - 1),
                            )
                        nc.vector.tensor_copy(
                            out=vt[:, d0 : d0 + dn], in_=ps[:, 0:dn]
                        )
                    vts.append(vt)

                for hd in range(NHEADS):
                    # scoresT [k, q] per kt; exp with mask-bias; P bf16
                    pts = []
                    sm = smps.tile([1, S], fp32, tag="sm")
                    for kt in range(NST):
                        sps = scps.tile([128, S], fp32, tag="sps")
                        nc.tensor.matmul(
                            sps[:],
                            qkt["k"][hd][:, kt * 128 : (kt + 1) * 128],
                            qkt["q"][hd][:],
                            start=True,
                            stop=True,
                        )
                        pt = pwork.tile([128, S], bf16, tag=f"pt{kt}")
                        nc.scalar.activation(
                            out=pt[:], in_=sps[:],
                            func=mybir.ActivationFunctionType.Exp,
                            bias=maskb[:, b * NST + kt : b * NST + kt + 1],
                            scale=SCALE,
                        )
                        pts.append(pt)
                        nc.tensor.matmul(
                            sm[:], ones1[:], pt[:],
                            start=(kt == 0), stop=(kt == NST - 1),
                        )
                    # ctxT [d, q]
                    cps = ctxps.tile([128, S], fp32, tag="cps")
                    for kt in range(NST):
                        nc.tensor.matmul(
                            cps[:],
                            vts[kt][:, hd * 128 : (hd + 1) * 128],
                            pts[kt][:],
                            start=(kt == 0),
                            stop=(kt == NST - 1),
                        )
                    rcp = pwork.tile([1, S], fp32, tag="rcp")
                    nc.vector.reciprocal(out=rcp[:], in_=sm[:])
                    rcpb = pwork.tile([1, S], bf16, tag="rcpb")
                    nc.vector.tensor_copy(out=rcpb[:], in_=rcp[:])
                    rb = rbps.tile([128, S], fp32, tag="rb")
                    nc.tensor.matmul(rb[:], onesr[:], rcpb[:], start=True, stop=True)
                    rbs = pwork.tile([128, S], fp32, tag="rbs")
                    nc.scalar.copy(out=rbs[:], in_=rb[:])
                    tmp = pwork.tile([128, S], fp32, tag="tmp")
                    nc.vector.tensor_tensor(
                        out=tmp[:], in0=cps[:], in1=rbs[:], op=mybir.AluOpType.mult
                    )
                    # fused residual + bv: ot = (ctx + bv) + h
                    ot = pwork.tile([128, S], fp32, tag="ot")
                    nc.vector.scalar_tensor_tensor(
                        out=ot[:], in0=tmp[:],
                        scalar=bqk[:, 2 * NDC + hd : 2 * NDC + hd + 1],
                        in1=hts[hd][:],
                        op0=mybir.AluOpType.add, op1=mybir.AluOpType.add,
                    )
                    # transpose [d, q] -> [q, d] per 128x128 tile and store
                    # out[b] as [S, D_ATT] so the host gets the final layout
                    for qt in range(NST):
                        tps = ctxps.tile([128, 128], fp32, tag="tps")
                        nc.tensor.transpose(
                            tps[:], ot[:, qt * 128 : (qt + 1) * 128], ident[:]
                        )
                        osb = pwork.tile([128, 128], fp32, tag="osb")
                        nc.scalar.copy(out=osb[:], in_=tps[:])
                        nc.sync.dma_start(
                            out=out_ext[
                                b, qt * 128 : (qt + 1) * 128,
                                hd * 128 : (hd + 1) * 128,
                            ],
                            in_=osb[:],
                        )
    nc.finalize()
    return nc


# ----------------------------------------------------- cached SPMD launcher
_EXEC_CACHE = {}


def _tlog(label, t0):
    import os, time, sys as _s
    if os.environ.get("KTIME"):
        print(f"[ktime] {label}: {time.time()-t0:.3f}s", file=_s.stderr, flush=True)
    return time.time()


def _get_exec(key, nc, n_cores):
    """Build (once) and cache a jitted shard_map executable for a Bass
    program.  run_bass_kernel_spmd re-creates the jit closure on every call,
    which forces a full retrace + XLA-cache round trip per launch; caching
    the jitted callable (same function object, same nc) makes warm calls
    dispatch in microseconds."""
    ent = _EXEC_CACHE.get(key)
    if ent is not None:
        return ent

    import jax
    import jax.numpy as jnp
    from jax.sharding import Mesh, PartitionSpec, NamedSharding
    from jax.experimental.shard_map import shard_map
    import concourse.mybir as mybir
    from concourse.bass2jax import (
        _bass_exec_p, install_neuronx_cc_hook, partition_id_tensor,
    )

    install_neuronx_cc_hook()
    assert nc.dbg_addr is None
    partition_name = (
        nc.partition_id_tensor.name if nc.partition_id_tensor else None
    )

    in_names, out_names, out_avals = [], [], []
    for alloc in nc.m.functions[0].allocations:
        if not isinstance(alloc, mybir.MemoryLocationSet):
            continue
        name = alloc.memorylocations[0].name
        if alloc.kind == "ExternalInput":
            if name != partition_name:
                in_names.append(name)
        elif alloc.kind == "ExternalOutput":
            out_names.append(name)
            out_avals.append(
                jax.core.ShapedArray(
                    tuple(alloc.tensor_shape), mybir.dt.np(alloc.dtype)
                )
            )
    n_params = len(in_names)
    n_outs = len(out_names)
    all_in = list(in_names) + list(out_names)
    if partition_name is not None:
        all_in.append(partition_name)
    all_in = tuple(all_in)
    donate = tuple(range(n_params, n_params + n_outs))

    def _body(*args):
        operands = list(args)
        if partition_name is not None:
            operands.append(partition_id_tensor())
        return tuple(
            _bass_exec_p.bind(
                *operands,
                out_avals=tuple(out_avals),
                in_names=all_in,
                out_names=tuple(out_names),
                lowering_input_output_aliases=(),
                sim_require_finite=True,
                sim_require_nnan=True,
                nc=nc,
            )
        )

    devices = jax.devices()[:n_cores]
    mesh = Mesh(np.asarray(devices), ("core",))
    spec = PartitionSpec("core")
    sharded = jax.jit(
        shard_map(
            _body,
            mesh=mesh,
            in_specs=(spec,) * (n_params + n_outs),
            out_specs=(spec,) * n_outs,
            check_rep=False,
        ),
        donate_argnums=donate,
        keep_unused=True,
    )
    zshapes = tuple(
        (n_cores * a.shape[0], *a.shape[1:]) for a in out_avals
    )
    zdtypes = tuple(a.dtype for a in out_avals)
    zero_fn = jax.jit(
        lambda: tuple(
            jnp.zeros(s, d) for s, d in zip(zshapes, zdtypes)
        ),
        out_shardings=tuple(NamedSharding(mesh, spec) for _ in out_avals),
    )
    ent = dict(
        sharded=sharded, zero_fn=zero_fn, in_names=in_names,
        out_names=out_names, out_avals=out_avals, n_cores=n_cores,
        mesh=mesh, spec=spec, dev_cache={},
    )
    _EXEC_CACHE[key] = ent
    return ent


def _dev_const(ent, name, builder):
    """Upload a replicated/static input once and reuse the device array."""
    arr = ent["dev_cache"].get(name)
    if arr is None:
        import jax
        from jax.sharding import NamedSharding

        arr = jax.device_put(
            builder(), NamedSharding(ent["mesh"], ent["spec"])
        )
        ent["dev_cache"][name] = arr
    return arr


def _run_spmd_dev(key, nc, n_cores, global_ins):
    """Launch a cached bass NEFF; inputs/outputs are GLOBAL jax arrays
    sharded P("core") (leading dim = n_cores * per-core dim).  Nothing
    touches the host."""
    ent = _get_exec(key, nc, n_cores)
    args = [global_ins[n] for n in ent["in_names"]]
    outs = ent["sharded"](*args, *ent["zero_fn"]())
    return dict(zip(ent["out_names"], outs))


def _mesh8():
    ent = _EXEC_CACHE.get("mesh8")
    if ent is None:
        import jax
        from jax.sharding import Mesh

        ent = Mesh(np.asarray(jax.devices()[:N_CORES]), ("core",))
        _EXEC_CACHE["mesh8"] = ent
    return ent


def _get_glue(name, build):
    fn = _EXEC_CACHE.get(("glue", name))
    if fn is None:
        fn = build()
        _EXEC_CACHE[("glue", name)] = fn
    return fn


def _build_glue1():
    """(xt12 flat [8,F] bf16, lens [8,16] i32) ->
    (xt [48,128,8192] bf16, vmask [1024,8192] bf16)."""
    import jax
    import jax.numpy as jnp
    from jax.sharding import PartitionSpec as P
    from jax.experimental.shard_map import shard_map

    mesh = _mesh8()

    def body(xt_s, lens_s):
        full = jax.lax.all_gather(xt_s, "core", axis=0, tiled=True)
        full = full.reshape(2 * (D_MODEL // 128), 128, S * B)
        pid = jax.lax.axis_index("core")
        kc = D_MODEL // 128
        my_xt = jnp.where(pid == 1, full[kc:], full[:kc])
        lens = lens_s[0]
        vmn = (jnp.arange(S, dtype=jnp.int32)[:, None] < lens[None, :]
               ).astype(jnp.bfloat16)
        vm = jnp.where(pid == 1, vmn[::-1], vmn).reshape(1, S * B)
        vmask = jnp.broadcast_to(vm, (128, S * B))
        return my_xt, vmask

    return jax.jit(shard_map(
        body, mesh=mesh, in_specs=(P("core"), P("core")),
        out_specs=(P("core"), P("core")), check_rep=False,
    ))


def _build_glue2():
    """(y [1024,40960] bf16, lens [8,16]) -> xt2 [80,128,8192] bf16.
    Masks invalid steps, concatenates directions, per-core time order."""
    import jax
    import jax.numpy as jnp
    from jax.sharding import PartitionSpec as P
    from jax.experimental.shard_map import shard_map

    mesh = _mesh8()

    def body(y_s, lens_s):
        y = jax.lax.all_gather(y_s, "core", axis=0, tiled=True)
        y = y.reshape(N_CORES, 128, S * 80)
        lens = lens_s[0]
        vm = (jnp.arange(S, dtype=jnp.int32)[:, None] < lens[None, :]
              ).astype(jnp.bfloat16)  # [S,B] natural
        yf = y[0].reshape(128, S, NKH, B) * vm[None, :, None, :]
        yb = y[1].reshape(128, S, NKH, B) * vm[::-1][None, :, None, :]

        def to_xt(a):  # [128,S,5,16] -> [5,128,S*16]
            return a.transpose(2, 0, 1, 3).reshape(NKH, 128, S * B)

        c0 = jnp.concatenate([to_xt(yf), to_xt(yb[:, ::-1])], axis=0)
        c1 = jnp.concatenate([to_xt(yf[:, ::-1]), to_xt(yb)], axis=0)
        pid = jax.lax.axis_index("core")
        return jnp.where(pid == 1, c1, c0)

    return jax.jit(shard_map(
        body, mesh=mesh, in_specs=(P("core"), P("core")),
        out_specs=P("core"), check_rep=False,
    ))


def _build_glue3():
    """(y2 [1024,40960] bf16, lens [8,16]) ->
    (ht [16,10,128,512] bf16, maskb [1024,8] f32)."""
    import jax
    import jax.numpy as jnp
    from jax.sharding import PartitionSpec as P
    from jax.experimental.shard_map import shard_map

    mesh = _mesh8()

    def body(y_s, lens_s):
        y = jax.lax.all_gather(y_s, "core", axis=0, tiled=True)
        y = y.reshape(N_CORES, 128, S * 80)
        lens = lens_s[0]
        vm = (jnp.arange(S, dtype=jnp.int32)[:, None] < lens[None, :]
              ).astype(jnp.bfloat16)
        yf = y[0].reshape(128, S, NKH, B) * vm[None, :, None, :]
        yb = (y[1].reshape(128, S, NKH, B) * vm[::-1][None, :, None, :])[:, ::-1]
        pid = jax.lax.axis_index("core")
        yf_l = jax.lax.dynamic_slice_in_dim(yf, 2 * pid, BPC, axis=3)
        yb_l = jax.lax.dynamic_slice_in_dim(yb, 2 * pid, BPC, axis=3)
        hf = yf_l.transpose(3, 2, 0, 1)  # [2,5,128,512]
        hb = yb_l.transpose(3, 2, 0, 1)
        ht = jnp.concatenate([hf, hb], axis=1)  # [2,10,128,512]
        lens_l = jax.lax.dynamic_slice_in_dim(lens, 2 * pid, BPC, axis=0)
        mk = jnp.where(
            jnp.arange(S, dtype=jnp.int32)[None, :] < lens_l[:, None],
            0.0, -10000.0,
        ).astype(jnp.float32)  # [2,512]
        maskb = mk.reshape(BPC, 4, 128).transpose(2, 0, 1).reshape(128, BPC * 4)
        return ht, maskb

    return jax.jit(shard_map(
        body, mesh=mesh, in_specs=(P("core"), P("core")),
        out_specs=(P("core"), P("core")), check_rep=False,
    ))


# ------------------------------------------------------------- host helpers
def _bf16(x):
    return np.ascontiguousarray(x.astype(BF16))


def _fp(*arrs):
    """Cheap fingerprint of source weight arrays for device-cache safety."""
    parts = []
    for a in arrs:
        a = np.asarray(a)
        flat = a.ravel()
        step = max(1, flat.size // 2048)
        parts.append((a.shape, a.dtype.str, hash(flat[::step].tobytes())))
    return tuple(parts)


def _pad8(arr2):
    """[2*d0, ...] real 2-core global -> [8*d0, ...] zero-padded."""
    d0 = arr2.shape[0] // 2
    out = np.zeros((N_CORES * d0, *arr2.shape[1:]), arr2.dtype)
    out[: 2 * d0] = arr2
    return out


def _lstm_weight_globals(Wih, Whh, bsum, kc_in):
    """Pack both directions' weights into zero-padded 8-core global arrays."""

    def pack(d):
        wiht = (Wih[d].T.reshape(kc_in, 128, H4).transpose(1, 0, 2)
                .reshape(128, kc_in * H4))
        whht = (Whh[d].T.reshape(NKH, 128, H4).transpose(1, 0, 2)
                .reshape(128, NKH * H4))
        bias = np.ascontiguousarray(
            bsum[d].reshape(NMC, 128).T.astype(np.float32))
        return _bf16(wiht), _bf16(whht), bias

    f = pack(0)
    b = pack(1)
    return dict(
        wiht=_pad8(np.concatenate([f[0], b[0]], axis=0)),
        whht=_pad8(np.concatenate([f[1], b[1]], axis=0)),
        bias=_pad8(np.concatenate([f[2], b[2]], axis=0)),
    )


def _dev_weights(ent, wkey, builder):
    dev = ent["dev_cache"].get(wkey)
    if dev is None:
        ent["dev_cache"].clear()
        import jax
        from jax.sharding import NamedSharding

        sh = NamedSharding(ent["mesh"], ent["spec"])
        dev = {k: jax.device_put(v, sh) for k, v in builder().items()}
        ent["dev_cache"][wkey] = dev
    return dev


def _device_forward(c_a_embeds, c_mask, c_lengths, Wih0, Whh0, b0, Wih1, Whh1,
                    b1, Wq, bq, Wk, bk, Wv, bv):
    import time
    import jax
    from jax.sharding import NamedSharding, PartitionSpec as P

    t0 = time.time()
    x = np.asarray(c_a_embeds, np.float32)
    lengths = np.asarray(c_lengths, np.int32)

    # build programs / executables (cached)
    for key, kc in (("lstm6", D_MODEL // 128), ("lstm10", D_ATT // 128)):
        if key not in _NC_CACHE:
            _NC_CACHE[key] = _build_lstm_nc(kc)
    if "attn" not in _NC_CACHE:
        _NC_CACHE["attn"] = _build_attn_nc()
    ent1 = _get_exec("lstm6", _NC_CACHE["lstm6"], N_CORES)
    ent2 = _get_exec("lstm10", _NC_CACHE["lstm10"], N_CORES)
    enta = _get_exec("attn", _NC_CACHE["attn"], N_CORES)
    glue1 = _get_glue("g1", _build_glue1)
    glue2 = _get_glue("g2", _build_glue2)
    glue3 = _get_glue("g3", _build_glue3)
    t = _tlog("setup", t0)

    # ---- host packing: x transposed both time orders, bf16
    kc1 = D_MODEL // 128
    xtf = x.transpose(2, 1, 0)  # [768, S, B] view
    xt12 = np.empty((2 * kc1, 128, S * B), BF16)
    xt12[:kc1] = xtf.astype(BF16).reshape(kc1, 128, S * B)
    xt12[kc1:] = xtf[:, ::-1, :].astype(BF16).reshape(kc1, 128, S * B)
    xt12_flat = xt12.reshape(N_CORES, -1)
    lens8 = np.ascontiguousarray(np.broadcast_to(lengths[None], (N_CORES, B)))
    t = _tlog("pack", t)

    # ---- weights (device-cached)
    w1 = _dev_weights(ent1, ("lstm_w", "lstm6", _fp(Wih0, Whh0, b0)),
                      lambda: _lstm_weight_globals(
                          np.asarray(Wih0, np.float32),
                          np.asarray(Whh0, np.float32),
                          np.asarray(b0, np.float32), kc1))
    w2 = _dev_weights(ent2, ("lstm_w", "lstm10", _fp(Wih1, Whh1, b1)),
                      lambda: _lstm_weight_globals(
                          np.asarray(Wih1, np.float32),
                          np.asarray(Whh1, np.float32),
                          np.asarray(b1, np.float32), D_ATT // 128))

    def attn_weights():
        def wt(W):
            return _bf16(np.asarray(W, np.float32).T
                         .reshape(NHEADS, 128, D_ATT).transpose(1, 0, 2)
                         .reshape(128, NHEADS * D_ATT))

        bqk = np.concatenate(
            [np.asarray(bq, np.float32).reshape(NHEADS, 128).T,
             np.asarray(bk, np.float32).reshape(NHEADS, 128).T,
             np.asarray(bv, np.float32).reshape(NHEADS, 128).T], axis=1)

        def rep(a):
            return np.ascontiguousarray(
                np.broadcast_to(a[None], (N_CORES, *a.shape))
            ).reshape(N_CORES * a.shape[0], *a.shape[1:])

        return dict(wqt=rep(wt(Wq)), wkt=rep(wt(Wk)), wvt=rep(wt(Wv)),
                    bqk=rep(bqk.astype(np.float32)))

    wa = _dev_weights(enta, ("attn_w", _fp(Wq, bq, Wk, bk, Wv, bv)),
                      attn_weights)
    t = _tlog("weights", t)

    # ---- device pipeline
    xt_g, vmask_g = glue1(xt12_flat, lens8)
    t = _tlog("glue1", t)
    y1 = _run_spmd_dev("lstm6", _NC_CACHE["lstm6"], N_CORES,
                       dict(xt=xt_g, vmask=vmask_g, **w1))["y"]
    t = _tlog("lstm6", t)
    xt2_g = glue2(y1, lens8)
    t = _tlog("glue2", t)
    y2 = _run_spmd_dev("lstm10", _NC_CACHE["lstm10"], N_CORES,
                       dict(xt=xt2_g, vmask=vmask_g, **w2))["y"]
    t = _tlog("lstm10", t)
    ht_g, maskb_g = glue3(y2, lens8)
    t = _tlog("glue3", t)
    out = _run_spmd_dev("attn", _NC_CACHE["attn"], N_CORES,
                        dict(ht=ht_g, maskb=maskb_g, **wa))["out"]
    t = _tlog("attn-dispatch", t)
    res = np.asarray(out)  # [16, 512, 1280] f32 -- the final answer
    t = _tlog("fetch", t)
    TRACE_LOG.append(("pipeline", None, round(time.time() - t0, 3)))
    return res


def kernel(c_a_embeds, c_mask, c_lengths, Wih0, Whh0, b0, Wih1, Whh1, b1,
           Wq, bq, Wk, bk, Wv, bv):
    try:
        out = _device_forward(c_a_embeds, c_mask, c_lengths, Wih0, Whh0, b0,
                              Wih1, Whh1, b1, Wq, bq, Wk, bk, Wv, bv)
    except Exception as e:  # pragma: no cover - fallback path
        import traceback
        traceback.print_exc()
        print(f"[kernel] device path failed ({type(e).__name__}: {e}); "
              "falling back to numpy", file=sys.stderr)
        out = _numpy_forward(c_a_embeds, c_mask, c_lengths, Wih0, Whh0, b0,
                             Wih1, Whh1, b1, Wq, bq, Wk, bk, Wv, bv)
    return np.ascontiguousarray(out.astype(np.float32))
